# revision 1
# baseline (speedup 1.0000x reference)
"""Trainium2 Bass kernel for the AttentionOptimizer problem.

Reference computation (B=2, L=20, N=8000):
    g  = grads.reshape(B, N);  gn = |g|
    d2[i,j]    = max(|pos_i|^2 + |pos_j|^2 - 2 pos_i.pos_j, 0)
    scores     = 2*(gn_i - gn_j) - 5*d2/L^2
    weights    = softmax_j(scores)
    g_smooth_i = sum_j weights[i,j] * g_j
    out        = spins - 0.05*(grads + 10*g_smooth) + noise

Key algebra used by the kernel: softmax is invariant to adding any
row-constant, so the `2*gn_i` and `-0.0125*|pos_i|^2` terms cancel in
weights.  The relu clamp on d2 only matters at |d2| ~ 1e-7 (score delta
~1e-9) and is dropped.  What remains is a pure attention kernel:

    weights[i,j] ∝ exp(0.025 * (pos_i . pos_j) + b_j)
    b_j = -2*gn_j - 0.0125*|pos_j|^2

The exp argument is computed entirely on the PE array as ONE bf16 matmul
with K=12: pos (scaled by sqrt(0.025)) split into bf16 hi+lo pairs
(recovers fp32 product precision; dropped lo*lo term < 3e-7), and b_j
split into three bf16 components streamed against constant-1 rows on the
i side (error < 1e-7).  Because K=12 uses only 12 of the PE's 128 rows,
the features are replicated into four 12-row bands at partitions
0/32/64/96 and each chunk's four 512-column matmuls are issued to
disjoint 32-row PE tiles (tile_position) — they execute concurrently,
~4x the naive throughput (this device pins the PE at 1.2 GHz).  The
single ScalarE Exp pass over each [128, 2048] PSUM tile needs no bias
operand, and its fused accum_out produces the softmax denominator for
free.  The numerator sum_j p[i,j]*g_j runs on the vector engine as
fused scalar_tensor_tensor multiply+accumulates against an fp16
broadcast of -0.5*g (the -0.5 = -LR*SMOOTH folds the final output
scaling in): half-row ops while the chain is still gated by ScalarE's
exp cadence (first NSPLIT i-blocks), then one full 8000-wide op per
i-block once the vector engine is the limiter.  The resulting DVE chain
runs gap-free and is the kernel's critical path (~140 us); ScalarE
finishes ~18 us earlier.

Sharding: 8 cores = 2 batches x 4 query-row quarters of 2000 rows
(padded to 2048).  Every core reads the full j-axis (padded to 8192 with
b_j = -1e5 so padded columns contribute exp() = 0 exactly); there is no
cross-core communication.  The i columns handed to each core are
permuted so that i_local = partition*16 + block, which makes the final
[128, 16] num/den tiles i-contiguous in DMA order (no transpose needed).

End-to-end numerical error vs the fp32 jax reference (numpy simulation
of every precision decision here): max abs err ~2e-6 on a ~4.2-absmax
output.
"""

import numpy as np
import ml_dtypes

import concourse.bacc as bacc
import concourse.mybir as mybir
import concourse.tile as tile
from concourse import bass_utils

BF16 = ml_dtypes.bfloat16

# Problem constants (hardcoded; kernel.py must be self-contained).
L = 20
B = 2
N = 8000          # L^3 lattice points
NP = 8192         # padded j extent (16 x 512)
Q = 4             # i-quarters per batch
IPC = 2000        # real i rows per core
IPAD = 2048       # padded i rows per core (16 blocks of 128)
NCORES = 8
JCHUNK = 2048     # j columns per PSUM tile (4 banks)
NJC = NP // JCHUNK
NIB = IPAD // 128
# Only the 8000 real j columns are processed; the last chunk is ragged
# (1856 wide) which trims ~2.3% off every engine's steady-state work.
JW = [JCHUNK, JCHUNK, JCHUNK, N - 3 * JCHUNK]
NSPLIT = 8        # i-blocks whose numerator runs as 2 half-row DVE ops
SCALE = np.float32(np.sqrt(0.025))   # pos prescale so t' = 0.025*pos.pos

_NC_CACHE = None
LAST_RESULTS = None  # BassKernelResults of the most recent run (for test.py)


def _build_program():
    """Build the (core-independent) Bass program once."""
    nc = bacc.Bacc("TRN2", target_bir_lowering=False, debug=False)
    dt = mybir.dt

    jfeat_d = nc.dram_tensor("jfeat", [12, NP], dt.bfloat16, kind="ExternalInput").ap()
    ifeat_d = nc.dram_tensor("ifeat", [12, IPAD], dt.bfloat16, kind="ExternalInput").ap()
    gb_d = nc.dram_tensor("gb", [128, NP], dt.float16, kind="ExternalInput").ap()
    sp_d = nc.dram_tensor("spins_s", [128, 16], dt.float32, kind="ExternalInput").ap()
    gr_d = nc.dram_tensor("grads_s", [128, 16], dt.float32, kind="ExternalInput").ap()
    no_d = nc.dram_tensor("noise_s", [128, 16], dt.float32, kind="ExternalInput").ap()
    out_d = nc.dram_tensor("out", [128, 16], dt.float32, kind="ExternalOutput").ap()

    with tile.TileContext(nc) as tc:
        with (
            tc.tile_pool(name="const", bufs=1) as cpool,
            tc.tile_pool(name="psum", bufs=1, space="PSUM") as ppool,
        ):
            # Replicate j/i features into four 12-row bands at partitions
            # 0/32/64/96: the K=12 matmuls then pack 4-at-a-time onto
            # disjoint 32-row PE groups (tile_position) and run
            # concurrently — ~4x PE throughput for this tiny-K shape.
            # DMA order = first-use order, split so the first compute chunk
            # unblocks after ~300 KB instead of the full ~3 MB of inputs.
            jf = cpool.tile([128, NP], dt.bfloat16)
            ift = cpool.tile([128, IPAD], dt.bfloat16)
            gbt = cpool.tile([128, NP], dt.float16)
            # Each HWDGE queue runs its transfers serially (~78 GB/s) and
            # each dma_start issue costs ~750 ns, so inputs are spread
            # over BOTH queues (SP + ACT) in first-use order.  The first
            # compute chunk (i-block 0, 2-way packed) needs only jf bands
            # 0/1 cols 0:2048, so those 49 KB slices go first.  The
            # startup is DMA-byte-bound: the ~1.56 MB that must precede
            # the first DVE op arrives at the same time under any
            # ordering (measured).
            for s in range(2):
                nc.sync.dma_start(out=ift[32 * s:32 * s + 12, :], in_=ifeat_d)
                nc.sync.dma_start(out=jf[32 * s:32 * s + 12, 0:JCHUNK],
                                  in_=jfeat_d[:, 0:JCHUNK])
            nc.sync.dma_start(out=gbt[:, JCHUNK:2 * JCHUNK],
                              in_=gb_d[:, JCHUNK:2 * JCHUNK])
            for s in range(2):
                nc.sync.dma_start(out=jf[32 * s:32 * s + 12, JCHUNK:N],
                                  in_=jfeat_d[:, JCHUNK:N])
            for s in range(2, 4):
                nc.scalar.dma_start(out=jf[32 * s:32 * s + 12, 0:N],
                                    in_=jfeat_d[:, 0:N])
            nc.scalar.dma_start(out=gbt[:, 0:JCHUNK], in_=gb_d[:, 0:JCHUNK])
            for s in range(2, 4):
                nc.scalar.dma_start(out=ift[32 * s:32 * s + 12, :], in_=ifeat_d)
            nc.scalar.dma_start(out=gbt[:, 2 * JCHUNK:3 * JCHUNK],
                                in_=gb_d[:, 2 * JCHUNK:3 * JCHUNK])
            nc.scalar.dma_start(out=gbt[:, 3 * JCHUNK:N],
                                in_=gb_d[:, 3 * JCHUNK:N])
            spt = cpool.tile([128, 16], dt.float32)
            nc.gpsimd.dma_start(out=spt[:], in_=sp_d)
            grt = cpool.tile([128, 16], dt.float32)
            nc.gpsimd.dma_start(out=grt[:], in_=gr_d)
            not_ = cpool.tile([128, 16], dt.float32)
            nc.gpsimd.dma_start(out=not_[:], in_=no_d)

            # First NSPLIT i-blocks contribute 2 num partials (cols
            # 2ib, 2ib+1); later blocks one (col NSPLIT + ib).
            num_parts = cpool.tile([128, NSPLIT + NIB], dt.float32)
            den_parts = cpool.tile([128, NIB * NJC], dt.float32)
            junk = cpool.tile([128, N], dt.float16)
            # p ring: 3 slots of one full 8000-wide i-block row each; the
            # numerator then needs only ONE fused multiply+accumulate per
            # i-block (16 instead of 32 DVE ops — less fixed overhead).
            pring = cpool.tile([128, 3 * N], dt.float16)

            # Dependency-free tiny Exp: pulls the ACT table load (~2.7us)
            # off the critical path.
            warm = cpool.tile([1, 16], dt.float32)
            nc.gpsimd.memset(warm[:], 0.0)
            nc.scalar.activation(warm[:], warm[:], mybir.ActivationFunctionType.Exp)

            # The slice-only part of the final combine depends just on the
            # input slices — emit it first so it runs in the DVE's idle
            # startup window instead of the post-chain tail:
            # tmp2 = (grads * -0.05 + spins) + noise.
            tmp = cpool.tile([128, NIB], dt.float32)
            tmp2 = cpool.tile([128, NIB], dt.float32)
            nc.vector.scalar_tensor_tensor(
                out=tmp[:],
                in0=grt[:],
                scalar=-0.05,
                in1=spt[:],
                op0=mybir.AluOpType.mult,
                op1=mybir.AluOpType.add,
            )
            nc.vector.tensor_add(tmp2[:], tmp[:], not_[:])

            # One persistent PSUM tensor covering all 8 banks; chunks
            # ping-pong between its two 4-bank halves.  (Separate pool
            # tiles made Tile emit 2 sync-waits on one Matmult, which the
            # MM ISA encoding cannot hold — bank-level deps within a
            # single tensor distribute the waits legally.)
            PT = ppool.tile([128, 2 * JCHUNK], dt.float32)
            ci = 0
            for ib in range(NIB):
                for jc in range(NJC):
                    w = JW[jc]
                    off = (ci % 2) * JCHUNK
                    # i-block 0 runs 2-way packed (bands 0/1 only) so its
                    # chunks start as soon as the first two jf band DMAs
                    # land; bands 2/3 stream in behind it.  All later
                    # blocks use the full 4-way concurrent packing.
                    ngrp = 2 if ib == 0 else 4
                    for s in range(4):
                        g = s % ngrp
                        c0 = jc * JCHUNK + s * 512
                        sw = min(512, w - s * 512)
                        nc.tensor.matmul(
                            PT[:, off + s * 512:off + s * 512 + sw],
                            lhsT=ift[32 * g:32 * g + 12, ib * 128:(ib + 1) * 128],
                            rhs=jf[32 * g:32 * g + 12, c0:c0 + sw],
                            start=True,
                            stop=True,
                            tile_position=(32 * g, 0),
                        )
                    slot = ib % 3
                    nc.scalar.activation(
                        pring[:, slot * N + jc * JCHUNK:slot * N + jc * JCHUNK + w],
                        PT[:, off:off + w],
                        mybir.ActivationFunctionType.Exp,
                        accum_out=den_parts[:, ci:ci + 1],
                    )
                    # Numerator multiply+accumulate on the DVE
                    # (tensor_tensor_reduce's raw ISA opcode crashes this
                    # device; scalar_tensor_tensor's fused accumulate is
                    # the working equivalent).  While the DVE chain is
                    # still gated by ScalarE's exp cadence (the first
                    # NSPLIT i-blocks), run half-row pieces so the DVE
                    # tracks ACT closely; once DVE-bound, one full
                    # 8000-wide op per i-block minimizes fixed overhead.
                    if ib < NSPLIT and jc % 2 == 1:
                        h0 = (jc - 1) * JCHUNK
                        hw = JW[jc - 1] + w
                        nc.vector.scalar_tensor_tensor(
                            out=junk[:, 0:hw],
                            in0=pring[:, slot * N + h0:slot * N + h0 + hw],
                            scalar=1.0,
                            in1=gbt[:, h0:h0 + hw],
                            op0=mybir.AluOpType.mult,
                            op1=mybir.AluOpType.mult,
                            accum_out=num_parts[:, 2 * ib + jc // 2:
                                                2 * ib + jc // 2 + 1],
                        )
                    elif ib >= NSPLIT and jc == NJC - 1:
                        nc.vector.scalar_tensor_tensor(
                            out=junk[:, 0:N],
                            in0=pring[:, slot * N:slot * N + N],
                            scalar=1.0,
                            in1=gbt[:, 0:N],
                            op0=mybir.AluOpType.mult,
                            op1=mybir.AluOpType.mult,
                            accum_out=num_parts[:, NSPLIT + ib:NSPLIT + ib + 1],
                        )
                    ci += 1

            den_all = cpool.tile([128, NIB], dt.float32)
            rden = cpool.tile([128, NIB], dt.float32)
            gsm = cpool.tile([128, NIB], dt.float32)
            outt = cpool.tile([128, NIB], dt.float32)

            nc.vector.tensor_reduce(
                den_all[:],
                den_parts[:].rearrange("p (i c) -> p i c", c=NJC),
                axis=mybir.AxisListType.X,
                op=mybir.AluOpType.add,
            )
            nc.vector.reciprocal(rden[:], den_all[:])
            num_final = cpool.tile([128, NIB], dt.float32)
            nc.vector.tensor_reduce(
                num_final[:, 0:NSPLIT],
                num_parts[:, 0:2 * NSPLIT].rearrange("p (i c) -> p i c", c=2),
                axis=mybir.AxisListType.X,
                op=mybir.AluOpType.add,
            )
            nc.vector.tensor_copy(out=num_final[:, NSPLIT:NIB],
                                  in_=num_parts[:, 2 * NSPLIT:NSPLIT + NIB])
            nc.vector.tensor_mul(gsm[:], num_final[:], rden[:])
            nc.vector.tensor_add(outt[:], tmp2[:], gsm[:])
            nc.sync.dma_start(out=out_d, in_=outt[:])

    nc.compile()
    return nc


def _host_prep(grads, spins, pos, noise):
    """Pure layout/format prep: shard, pad, transpose, dtype-split."""
    f32 = np.float32
    g = np.ascontiguousarray(grads, dtype=f32).reshape(B, N)
    gn = np.abs(g)
    pos32 = np.ascontiguousarray(pos, dtype=f32)
    sq = (pos32 * pos32).sum(-1, dtype=f32)
    b = (-2.0 * gn - 0.0125 * sq[None, :]).astype(f32)  # [B, N]

    posS = (pos32 * SCALE).astype(f32)
    hi = posS.astype(BF16)
    lo = (posS - hi.astype(f32)).astype(BF16)
    b1 = b.astype(BF16)
    r = (b - b1.astype(f32)).astype(f32)
    b2 = r.astype(BF16)
    b3 = (r - b2.astype(f32)).astype(BF16)

    # jfeat per batch: [12, NP] bf16
    jfeat = np.zeros((B, 12, NP), BF16)
    jfeat[:, 0:3, :N] = hi.T[None]
    jfeat[:, 3:6, :N] = lo.T[None]
    jfeat[:, 6:9, :N] = hi.T[None]
    jfeat[:, 9, :N] = b1
    jfeat[:, 10, :N] = b2
    jfeat[:, 11, :N] = b3
    jfeat[:, 9, N:] = BF16(-1e5)  # padded j columns: exp(...) == 0 exactly

    # gbcast per batch: [128, NP] fp16 of -0.5*g (the -LR*SMOOTH fold)
    gb = np.zeros((B, 128, NP), np.float16)
    gb[:, :, :N] = (-0.5 * g).astype(np.float16)[:, None, :]

    # i-column permutation: col c <-> i_local = (c % 128) * 16 + c // 128
    cols = np.arange(IPAD)
    il = (cols % 128) * 16 + cols // 128  # i_local for each ifeat column

    spins_f = np.ascontiguousarray(spins, dtype=f32).reshape(B, N)
    noise_f = np.ascontiguousarray(noise, dtype=f32).reshape(B, N)

    in_maps = []
    for core in range(NCORES):
        bi, q = divmod(core, Q)
        gi = q * IPC + il  # global i index per ifeat column
        valid = il < IPC

        ifeat = np.zeros((12, IPAD), BF16)
        gi_v = gi[valid]
        ifeat[0:3, valid] = hi.T[:, gi_v]
        ifeat[3:6, valid] = hi.T[:, gi_v]
        ifeat[6:9, valid] = lo.T[:, gi_v]
        ifeat[9:12, :] = BF16(1.0)

        def slice_pad(x):
            s = np.zeros(IPAD, f32)
            s[:IPC] = x[bi, q * IPC:(q + 1) * IPC]
            return s.reshape(128, 16)  # [p, ib] with i_local = p*16 + ib

        in_maps.append({
            "jfeat": np.ascontiguousarray(jfeat[bi]),
            "ifeat": ifeat,
            "gb": np.ascontiguousarray(gb[bi]),
            "spins_s": slice_pad(spins_f),
            "grads_s": slice_pad(g),
            "noise_s": slice_pad(noise_f),
        })
    return in_maps


def kernel(grads, spins, pos, noise, trace=False, **run_kwargs):
    global _NC_CACHE, LAST_RESULTS
    if _NC_CACHE is None:
        _NC_CACHE = _build_program()
    nc = _NC_CACHE

    in_maps = _host_prep(grads, spins, pos, noise)
    res = bass_utils.run_bass_kernel_spmd(
        nc, in_maps, core_ids=list(range(NCORES)), trace=trace, **run_kwargs
    )
    LAST_RESULTS = res

    out = np.empty((B, N), np.float32)
    for core in range(NCORES):
        bi, q = divmod(core, Q)
        o = np.asarray(res.results[core]["out"], dtype=np.float32).reshape(IPAD)
        out[bi, q * IPC:(q + 1) * IPC] = o[:IPC]
    return out.reshape(B, L, L, L)



# revision 3
# speedup vs baseline: 6.9714x; 6.9714x over previous
"""Trainium2 Bass kernel for the AttentionOptimizer problem.

Reference computation (B=2, L=20, N=8000):
    g  = grads.reshape(B, N);  gn = |g|
    d2[i,j]    = max(|pos_i|^2 + |pos_j|^2 - 2 pos_i.pos_j, 0)
    scores     = 2*(gn_i - gn_j) - 5*d2/L^2
    weights    = softmax_j(scores)
    g_smooth_i = sum_j weights[i,j] * g_j
    out        = spins - 0.05*(grads + 10*g_smooth) + noise

Key algebra: softmax drops row-constants, so
    weights[i,j] ∝ exp(b_j + t_ij),  b_j = -2|g_j| - 0.0125|p_j|^2,
    t_ij = 0.025 * (pos_i . pos_j)  with  |t| <= 0.025*3 = 0.075.
Because |t| is tiny, exp(t) is replaced by its 2nd-order Taylor
polynomial P2(t) = 1 + t + t^2/2 (rel. weight error <= |t|^3/6*e^|t|
~ 7e-5, and the signed errors average out inside the j-sums: the
end-to-end fp32 error vs the jax reference is ~5.6e-8 relative —
identical to evaluating exp() exactly, i.e. at the reference's own
fp32 noise floor; validated in a bit-faithful numpy pipeline sim).

P2(t) factorizes over a 10-term monomial basis m(p) =
[1, x, y, z, x2, y2, z2, xy, xz, yz]:
    P2(t_ij) = Phi(p_i) . m(p_j),
    Phi = [1, .025x, .025y, .025z, c x2, c y2, c z2, 2c xy, 2c xz, 2c yz],
    c = 0.025^2/2.
So the whole attention collapses to 20 weighted j-moments
    S_m = sum_j w_j m_m(p_j),   G_m = sum_j w_j g_j m_m(p_j)
and a per-i quadratic evaluation
    g_smooth_i = (Phi_i . G) / (Phi_i . S).

Device mapping (everything fp32):
  * j axis (8000, padded to 8064) lives as [128, 63] tiles.  DVE
    computes b_j, then a chain of fused scalar_tensor_tensor multiplies
    with accum_out produces all 20 per-partition moment partials
    (S0 comes free from the Exp accum_out, G0 from the w*g product).
  * One fp32 matmul vs a ones-vector reduces partials across the 128
    partitions into a [1,20] PSUM row; a second K=1 matmul broadcasts
    that row back to all 128 partitions ([128,20]).
  * i axis: each core owns 2000 rows as [128,16] (i = p*16 + c).  The
    den/num polynomials are evaluated as 2x10 DVE multiply-add chains
    using the per-partition-scalar operand of scalar_tensor_tensor
    (broadcast along free), then reciprocal/mult/final-combine.
  * The S-half of the moment pipeline (reduce mm + PSUM copy +
    broadcast mm) runs while the DVE is still accumulating G-moments,
    so the den chain starts with no tail stall.

Sharding: 8 cores = 2 batches x 4 query-quarters of 2000 i rows.  Each
core recomputes the (tiny) j-moment phase for its batch; there is no
cross-core communication.  Inputs per core: pos coords [128,189], g
[128,63], Phi features [128,160], spins|grads|noise slices [128,48] —
~230 KB, split over both HWDGE queues.
"""

import numpy as np

import concourse.bacc as bacc
import concourse.mybir as mybir
import concourse.tile as tile
from concourse import bass_utils

# Problem constants (hardcoded; kernel.py must be self-contained).
L = 20
B = 2
N = 8000          # L^3 lattice points
JP = 128 * 63     # padded j extent (8064)
JC = 63           # j columns per partition
Q = 4             # i-quarters per batch
IPC = 2000        # real i rows per core
IPAD = 2048       # padded i rows per core ([128, 16])
NCORES = 8
GAMMA = np.float32(0.025)
C2 = np.float32(0.025 * 0.025 / 2.0)

_NC_CACHE = None
LAST_RESULTS = None  # BassKernelResults of the most recent run (for test.py)


def _build_program():
    nc = bacc.Bacc("TRN2", target_bir_lowering=False, debug=False)
    dt = mybir.dt
    f32 = dt.float32
    Alu = mybir.AluOpType
    Act = mybir.ActivationFunctionType

    posc_d = nc.dram_tensor("posc", [128, 3 * JC], f32, kind="ExternalInput").ap()
    gj_d = nc.dram_tensor("gj", [128, JC], f32, kind="ExternalInput").ap()
    phi_d = nc.dram_tensor("phi", [128, 160], f32, kind="ExternalInput").ap()
    sgn_d = nc.dram_tensor("sgn", [128, 48], f32, kind="ExternalInput").ap()
    out_d = nc.dram_tensor("out", [128, 16], f32, kind="ExternalOutput").ap()

    with tile.TileContext(nc) as tc:
        with (
            tc.tile_pool(name="const", bufs=1) as cpool,
            tc.tile_pool(name="psum", bufs=1, space="PSUM") as ppool,
        ):
            posc = cpool.tile([128, 3 * JC], f32)
            gj = cpool.tile([128, JC], f32)
            phi = cpool.tile([128, 160], f32)
            sgn = cpool.tile([128, 48], f32)
            # Input DMAs split across both HWDGE queues in first-use
            # order: gj (ACT queue) feeds the first DVE ops, posc (SP
            # queue) the monomials; phi is only needed ~4us in.
            nc.scalar.dma_start(out=gj[:], in_=gj_d)
            nc.sync.dma_start(out=posc[:], in_=posc_d)
            nc.scalar.dma_start(out=sgn[:], in_=sgn_d)
            nc.sync.dma_start(out=phi[:], in_=phi_d)

            ones128 = cpool.tile([128, 1], f32)
            ones1t = cpool.tile([1, 128], f32)
            nc.gpsimd.memset(ones128[:], 1.0)
            nc.gpsimd.memset(ones1t[:], 1.0)

            # Dependency-free tiny Exp pulls the ACT table load (~2.7us)
            # off the critical path.
            warm = cpool.tile([1, 16], f32)
            nc.gpsimd.memset(warm[:], 0.0)
            nc.scalar.activation(warm[:], warm[:], Act.Exp)

            px = posc[:, 0:JC]
            py = posc[:, JC:2 * JC]
            pz = posc[:, 2 * JC:3 * JC]

            # b_j = -2|g_j| - 0.0125|p_j|^2, built on DVE (no ACT Abs —
            # avoids any activation-table-set switch).
            m2g = cpool.tile([128, JC], f32)
            tng = cpool.tile([128, JC], f32)
            nc.vector.tensor_scalar_mul(m2g[:], gj[:], -2.0)
            nc.vector.scalar_tensor_tensor(
                out=tng[:], in0=gj[:], scalar=2.0, in1=m2g[:],
                op0=Alu.mult, op1=Alu.min)  # min(2g, -2g) = -2|g|

            # tmp2 = (grads*-0.05 + spins) + noise runs in the DMA window.
            tmp = cpool.tile([128, 16], f32)
            tmp2 = cpool.tile([128, 16], f32)
            nc.vector.scalar_tensor_tensor(
                out=tmp[:], in0=sgn[:, 16:32], scalar=-0.05,
                in1=sgn[:, 0:16], op0=Alu.mult, op1=Alu.add)
            nc.vector.tensor_add(tmp2[:], tmp[:], sgn[:, 32:48])

            sq3 = cpool.tile([128, 3 * JC], f32)
            ssq = cpool.tile([128, JC], f32)
            bt = cpool.tile([128, JC], f32)
            nc.vector.tensor_mul(sq3[:], posc[:], posc[:])
            nc.vector.tensor_reduce(
                ssq[:],
                sq3[:].rearrange("p (k c) -> p c k", k=3),
                axis=mybir.AxisListType.X,
                op=Alu.add)
            nc.vector.scalar_tensor_tensor(
                out=bt[:], in0=ssq[:], scalar=-0.0125, in1=tng[:],
                op0=Alu.mult, op1=Alu.add)

            # partials cols: 0..9 = S-moments, 10..19 = G-moments, in
            # basis order [1, x, y, z, xx, yy, zz, xy, xz, yz].
            partials = cpool.tile([128, 20], f32)
            w = cpool.tile([128, JC], f32)
            nc.scalar.activation(w[:], bt[:], Act.Exp,
                                 accum_out=partials[:, 0:1])  # S0

            wg = cpool.tile([128, JC], f32)
            wx = cpool.tile([128, JC], f32)
            wy = cpool.tile([128, JC], f32)
            wz = cpool.tile([128, JC], f32)
            ux = cpool.tile([128, JC], f32)
            uy = cpool.tile([128, JC], f32)
            uz = cpool.tile([128, JC], f32)
            junk = cpool.tile([128, JC], f32)

            def macc(out_t, in0, in1, col):
                nc.vector.scalar_tensor_tensor(
                    out=out_t, in0=in0, scalar=1.0, in1=in1,
                    op0=Alu.mult, op1=Alu.mult,
                    accum_out=partials[:, col:col + 1])

            # S-half first so its reduce/broadcast runs under the
            # G-moment DVE ops.
            macc(wx[:], w[:], px, 1)
            macc(wy[:], w[:], py, 2)
            macc(wz[:], w[:], pz, 3)
            macc(junk[:], wx[:], px, 4)   # Sxx
            macc(junk[:], wy[:], py, 5)   # Syy
            macc(junk[:], wz[:], pz, 6)   # Szz
            macc(junk[:], wx[:], py, 7)   # Sxy
            macc(junk[:], wx[:], pz, 8)   # Sxz
            macc(junk[:], wy[:], pz, 9)   # Syz

            PT = ppool.tile([128, 40], f32)
            rrow = cpool.tile([1, 20], f32)
            rb = cpool.tile([128, 20], f32)

            # S: reduce partials over partitions -> [1,10] row, then
            # broadcast the row to all 128 partitions.
            nc.tensor.matmul(PT[0:1, 20:30], lhsT=ones128[:],
                             rhs=partials[:, 0:10], start=True, stop=True)
            nc.scalar.activation(rrow[0:1, 0:10], PT[0:1, 20:30], Act.Copy)
            nc.tensor.matmul(PT[:, 0:10], lhsT=ones1t[:],
                             rhs=rrow[0:1, 0:10], start=True, stop=True)
            nc.scalar.activation(rb[:, 0:10], PT[:, 0:10], Act.Copy)

            macc(wg[:], w[:], gj[:], 10)  # G0
            macc(ux[:], wg[:], px, 11)
            macc(uy[:], wg[:], py, 12)
            macc(uz[:], wg[:], pz, 13)
            macc(junk[:], ux[:], px, 14)  # Gxx
            macc(junk[:], uy[:], py, 15)  # Gyy
            macc(junk[:], uz[:], pz, 16)  # Gzz
            macc(junk[:], ux[:], py, 17)  # Gxy
            macc(junk[:], ux[:], pz, 18)  # Gxz
            macc(junk[:], uy[:], pz, 19)  # Gyz

            nc.tensor.matmul(PT[0:1, 30:40], lhsT=ones128[:],
                             rhs=partials[:, 10:20], start=True, stop=True)
            nc.scalar.activation(rrow[0:1, 10:20], PT[0:1, 30:40], Act.Copy)
            nc.tensor.matmul(PT[:, 10:20], lhsT=ones1t[:],
                             rhs=rrow[0:1, 10:20], start=True, stop=True)
            nc.scalar.activation(rb[:, 10:20], PT[:, 10:20], Act.Copy)

            # den/num polynomial chains: acc = phi_m * R_m + acc, with
            # R_m broadcast per-partition via the scalar operand.
            acc = [cpool.tile([128, 16], f32, name=f"acc{k}")
                   for k in range(4)]

            def chain(base):
                cur = None
                for m in range(10):
                    pm = phi[:, 16 * m:16 * (m + 1)]
                    sc = rb[:, base + m:base + m + 1]
                    if cur is None:
                        cur = acc[2 * (base // 10)]
                        nc.vector.tensor_scalar(
                            out=cur[:], in0=pm, scalar1=sc, scalar2=None,
                            op0=Alu.mult)
                    else:
                        nxt = acc[2 * (base // 10) + (m % 2)]
                        nc.vector.scalar_tensor_tensor(
                            out=nxt[:], in0=pm, scalar=sc, in1=cur[:],
                            op0=Alu.mult, op1=Alu.add)
                        cur = nxt
                return cur

            den = chain(0)    # overlaps the G-half matmul/copy tail
            num = chain(10)

            rden = cpool.tile([128, 16], f32)
            gsm = cpool.tile([128, 16], f32)
            outt = cpool.tile([128, 16], f32)
            nc.vector.reciprocal(rden[:], den[:])
            nc.vector.tensor_mul(gsm[:], num[:], rden[:])
            nc.vector.scalar_tensor_tensor(
                out=outt[:], in0=gsm[:], scalar=-0.5, in1=tmp2[:],
                op0=Alu.mult, op1=Alu.add)
            nc.sync.dma_start(out=out_d, in_=outt[:])

    nc.compile()
    return nc


def _host_prep(grads, spins, pos, noise):
    """Layout/format prep: shard, pad, monomial features."""
    f32 = np.float32
    g = np.ascontiguousarray(grads, dtype=f32).reshape(B, N)
    spins_f = np.ascontiguousarray(spins, dtype=f32).reshape(B, N)
    noise_f = np.ascontiguousarray(noise, dtype=f32).reshape(B, N)
    pos32 = np.ascontiguousarray(pos, dtype=f32)

    # j-side tiles (j = p*63 + c); pad j>=N with pos=0, g=1e9 (w=0).
    def jpad(v, fill):
        a = np.full(JP, fill, f32)
        a[:N] = v
        return a.reshape(128, JC)

    posc = np.concatenate(
        [jpad(pos32[:, 0], 0.0), jpad(pos32[:, 1], 0.0),
         jpad(pos32[:, 2], 0.0)], axis=1)
    gj = [jpad(g[bi], 1e9) for bi in range(B)]

    # i-side Phi features per quarter: [128, 10*16], i = p*16 + c.
    phis = []
    for q in range(Q):
        gi = np.clip(q * IPC + np.arange(IPAD), 0, N - 1)
        valid = np.arange(IPAD) < IPC
        X, Y, Z = pos32[gi, 0], pos32[gi, 1], pos32[gi, 2]
        P = np.zeros((10, IPAD), f32)
        P[0] = 1.0
        P[1], P[2], P[3] = GAMMA * X, GAMMA * Y, GAMMA * Z
        P[4], P[5], P[6] = C2 * X * X, C2 * Y * Y, C2 * Z * Z
        P[7], P[8], P[9] = 2 * C2 * X * Y, 2 * C2 * X * Z, 2 * C2 * Y * Z
        P[:, ~valid] = 0.0
        P[0, ~valid] = 1.0  # keep den = S0 on pad rows (finite)
        phis.append(np.ascontiguousarray(
            P.reshape(10, 128, 16).transpose(1, 0, 2).reshape(128, 160)))

    def sl(x, bi, q):
        s = np.zeros(IPAD, f32)
        s[:IPC] = x[bi, q * IPC:(q + 1) * IPC]
        return s.reshape(128, 16)

    in_maps = []
    for core in range(NCORES):
        bi, q = divmod(core, Q)
        sgn = np.concatenate(
            [sl(spins_f, bi, q), sl(g, bi, q), sl(noise_f, bi, q)], axis=1)
        in_maps.append({
            "posc": posc,
            "gj": gj[bi],
            "phi": phis[q],
            "sgn": np.ascontiguousarray(sgn),
        })
    return in_maps


def kernel(grads, spins, pos, noise, trace=False, **run_kwargs):
    global _NC_CACHE, LAST_RESULTS
    if _NC_CACHE is None:
        _NC_CACHE = _build_program()
    nc = _NC_CACHE

    in_maps = _host_prep(grads, spins, pos, noise)
    res = bass_utils.run_bass_kernel_spmd(
        nc, in_maps, core_ids=list(range(NCORES)), trace=trace, **run_kwargs
    )
    LAST_RESULTS = res

    out = np.empty((B, N), np.float32)
    for core in range(NCORES):
        bi, q = divmod(core, Q)
        o = np.asarray(res.results[core]["out"], dtype=np.float32).reshape(IPAD)
        out[bi, q * IPC:(q + 1) * IPC] = o[:IPC]
    return out.reshape(B, L, L, L)
